# revision 23
# baseline (speedup 1.0000x reference)
"""Trainium2 Bass kernel for nn_MAB_44057774522768 (Set-Transformer MAB block).

Reference computation (per batch b, with B=8, Sq=Sk=1024, D=512, H=8 heads,
dh=64):
    Qp = Q @ Wq.T + bq                  [Sq, D]
    Kp = K @ Wk.T + bk                  [Sk, D]
    Vp = K @ Wv.T + bv                  [Sk, D]
    scores_h = Qp_h @ Kp_h.T / sqrt(D)  per head  [Sq, Sk]
    A = softmax(scores, axis=-1)
    ctx_h = A_h @ Vp_h
    O1 = Qp + ctx                       (residual on projected Q)
    out = O1 + relu(O1 @ Wo.T + bo)     (FFN residual)

Sharding: pure data-parallel, batch b -> core b (B == 8 == n_cores).

Device-side layout: "feature-major" — activations stored transposed
[feature, seq] so every matmul contracts over the partition axis with zero
on-chip transposes:
    QpT = (Wq.T).T @ QT   (fp32r matmuls, ~2e-4 rel err, full PE rate)
    scoresT_h[k, q] = KpT_h.T @ QpT_h  (bf16, K=64 row-packed pairs of heads)
    expT = exp(scoresT / sqrt(512))    (ACT, bf16 out; no max-subtraction
                                        needed: |scores/sqrt(512)| < ~1)
    ctxT_h = [Vp_h | 1].T @ expT_h     (bf16; ones column makes row 64 of the
                                        PSUM accumulator the softmax
                                        denominator — no separate reduction)
    denominators of a pair -> DRAM -> gather-transposed to [128,16] ->
    128-lane reciprocal -> scatter-transposed back -> partition-broadcast
    DMA into rb;  ctx = ctxU * rb
    O1T = QpT + ctxT;  outT = O1T + bv + relu(WoT.T @ O1T + bo + Wo@bv)

bv is NOT added to Vp on-device: softmax weights sum to 1, so A @ (Vp + bv)
== A @ Vp + bv; bv is added at the end and folded into the FFN bias
(bo2 = bo + Wo @ bv) on the host.

Scores use bf16 operands: with full-fp32r scores, the two-pass fp32r
projection matmuls interleaved between row-packed score matmuls corrupt
PSUM on hardware (single-pass bf16 scores are immune, and the extra error
is negligible next to the bf16 exp output).
"""

import math
import os

import numpy as np

import concourse.bass as bass
import concourse.mybir as mybir
import concourse.tile as tile
from concourse import bacc
from concourse.bass_utils import run_bass_kernel_spmd

B, SQ, SK, D = 8, 1024, 1024, 512
H, DH = 8, 64
N_CORES = 8
KC = D // 128  # 4 contraction chunks of 128 (din)
MT = D // 128  # 4 output-feature tiles of 128 (dout)
NQ = SQ // 512  # 2 moving chunks of 512 (seq)
KT8 = SK // 128  # 8 key-seq tiles of 128

F32R = mybir.dt.float32r
F32 = mybir.dt.float32
BF16 = mybir.dt.bfloat16
ALU = mybir.AluOpType
ACTF = mybir.ActivationFunctionType

_NC = None


def _build():
    nc = bacc.Bacc(None, target_bir_lowering=False, debug=False)

    dQT = nc.dram_tensor("QT", [D, SQ], F32R, kind="ExternalInput")
    dKT = nc.dram_tensor("KT", [D, SK], BF16, kind="ExternalInput")
    dWq = nc.dram_tensor("WqT", [D, D], F32R, kind="ExternalInput")  # [din,dout]
    dWk = nc.dram_tensor("WkT", [D, D], BF16, kind="ExternalInput")
    dWv = nc.dram_tensor("WvT", [D, D], BF16, kind="ExternalInput")
    dWo = nc.dram_tensor("WoT", [D, D], F32R, kind="ExternalInput")
    dBQ = nc.dram_tensor("BQ", [128, MT], F32, kind="ExternalInput")
    dBK = nc.dram_tensor("BK", [128, MT], F32, kind="ExternalInput")
    dBO2 = nc.dram_tensor("BO2", [128, MT], F32, kind="ExternalInput")
    dBV = nc.dram_tensor("BV", [128, MT], F32, kind="ExternalInput")
    dOT = nc.dram_tensor("OT", [D, SQ], F32, kind="ExternalOutput")

    dbg = os.environ.get("KDEBUG", "0") == "1"
    if dbg:
        dDQP = nc.dram_tensor("DQP", [128, MT, SQ], F32, kind="ExternalOutput")
        dDKP = nc.dram_tensor("DKP", [128, MT, SK], BF16, kind="ExternalOutput")
        dDVPA = nc.dram_tensor(
            "DVPA", [128, KT8, H, DH + 1], BF16, kind="ExternalOutput"
        )
        dDEX = nc.dram_tensor("DEX", [128, SQ], BF16, kind="ExternalOutput")
        dDRB = nc.dram_tensor("DRB", [128, SQ], F32, kind="ExternalOutput")
        dDCUA = nc.dram_tensor("DCUA", [128, SQ], F32, kind="ExternalOutput")
        dDO1 = nc.dram_tensor("DO1", [128, MT, SQ], F32, kind="ExternalOutput")

    scale = 1.0 / math.sqrt(float(D))

    with tile.TileContext(nc) as tc:
        with (
            tc.tile_pool(name="persist", bufs=1) as persist,
            tc.tile_pool(name="ppool", bufs=2, space="PSUM") as ppool,
            tc.tile_pool(name="spool", bufs=2, space="PSUM") as spool,
            tc.tile_pool(name="cpool", bufs=1, space="PSUM") as cpool,
            tc.tile_pool(name="epool", bufs=3) as epool,
            tc.tile_pool(name="pairpool", bufs=2) as pairpool,
            tc.tile_pool(name="smallpool", bufs=3) as smallpool,
            tc.tile_pool(name="outpool", bufs=3) as outpool,
            tc.tile_pool(name="dpool", bufs=2, space="DRAM") as dpool,
        ):
            # ---- persistent SBUF tensors ----
            qt = persist.tile([128, KC, SQ], F32R)
            kt = persist.tile([128, KC, SK], BF16)
            wq = persist.tile([128, KC, D], F32R)
            wk = persist.tile([128, KC, D], BF16)
            wv = persist.tile([128, KC, D], BF16)
            wo = persist.tile([128, KC, D], F32R)
            bq = persist.tile([128, MT], F32)
            bk = persist.tile([128, MT], F32)
            bo2 = persist.tile([128, MT], F32)
            bv = persist.tile([128, MT], F32)
            qp = persist.tile([128, MT, SQ], F32R)
            qpb = persist.tile([128, MT, SQ], BF16)
            kpb = persist.tile([128, MT, SK], BF16)
            # Vp in seq-major [k, h, dh] + ones column at dh=64 per head
            vpa = persist.tile([128, KT8, H, DH + 1], BF16)
            o1 = persist.tile([128, MT, SQ], F32R)

            # ---- input DMAs, ordered by first use; split across the two
            # HWDGE queues (SP + Activation) so transfers overlap ----
            nc.sync.dma_start(out=bq, in_=dBQ[:, :])
            nc.sync.dma_start(out=bk, in_=dBK[:, :])
            nc.sync.dma_start(out=bo2, in_=dBO2[:, :])
            nc.sync.dma_start(out=bv, in_=dBV[:, :])
            for kc in range(KC):
                nc.sync.dma_start(out=wq[:, kc, :], in_=dWq[kc * 128:(kc + 1) * 128, :])
                nc.scalar.dma_start(out=qt[:, kc, :], in_=dQT[kc * 128:(kc + 1) * 128, :])
            for kc in range(KC):
                nc.sync.dma_start(out=wk[:, kc, :], in_=dWk[kc * 128:(kc + 1) * 128, :])
                nc.scalar.dma_start(out=kt[:, kc, :], in_=dKT[kc * 128:(kc + 1) * 128, :])
            for kc in range(KC):
                nc.sync.dma_start(out=wv[:, kc, :], in_=dWv[kc * 128:(kc + 1) * 128, :])
            for kc in range(KC):
                nc.gpsimd.dma_start(out=wo[:, kc, :], in_=dWo[kc * 128:(kc + 1) * 128, :])

            # ones column for the fused softmax denominator
            nc.vector.memset(vpa[:, :, :, DH:DH + 1], 1.0)

            def project(dst, w, rhs_src, bias_ap, m, n, dst2=None):
                """dst[:, m, nsl] = (w[:,:,m-tile].T @ rhs_src[:,:,nsl]) + bias."""
                nsl = slice(n * 512, (n + 1) * 512)
                pp = ppool.tile([128, 512], F32, name="pp", tag="pp")
                for kc in range(KC):
                    nc.tensor.matmul(
                        pp[:, :],
                        w[:, kc, m * 128:(m + 1) * 128],
                        rhs_src[:, kc, nsl],
                        start=(kc == 0),
                        stop=(kc == KC - 1),
                    )
                nc.vector.tensor_scalar(dst[:, m, nsl], pp[:, :], bias_ap, None, ALU.add)
                if dst2 is not None:
                    nc.vector.tensor_scalar(
                        dst2[:, m, nsl], pp[:, :], bias_ap, None, ALU.add
                    )

            def project_v(mt):
                """vpa[:, mt, h, 0:64] = (kt[:,:,mt-tile].T @ wv) in bf16."""
                pv = ppool.tile([128, 512], F32, name="pv", tag="pp")
                for kc in range(KC):
                    nc.tensor.matmul(
                        pv[:, :],
                        kt[:, kc, mt * 128:(mt + 1) * 128],
                        wv[:, kc, :],
                        start=(kc == 0),
                        stop=(kc == KC - 1),
                    )
                nc.vector.tensor_copy(
                    vpa[:, mt, :, 0:DH],
                    pv[:, :].rearrange("p (h d) -> p h d", h=H),
                )

            # deferred projection chunks, drip-fed into attention pairs so the
            # PE fills its exp-wait gaps without delaying the pair boundary
            fillers = []
            for t in range(1, 4):
                for n in range(NQ):
                    fillers.append(
                        lambda t=t, n=n: project(
                            qp, wq, qt, bq[:, t:t + 1], t, n, dst2=qpb
                        )
                    )
                for n in range(NQ):
                    fillers.append(
                        lambda t=t, n=n: project(kpb, wk, kt, bk[:, t:t + 1], t, n)
                    )

            def attend_pair(t, fill_budget):
                """Heads 2t (partitions 0-63) and 2t+1 (64-127)."""
                cua = pairpool.tile([128, SQ], F32, name="cua", tag="cua")
                rb = pairpool.tile([128, SQ], F32, name="rb", tag="rb")
                nfill = 0
                for hh in range(2):
                    h = 2 * t + hh
                    hb = 64 * hh
                    pc = cpool.tile([DH + 1, SQ], F32, name="pc", tag="pc")
                    for m in range(KT8):
                        ps = spool.tile([128, SQ], F32, name="ps", tag="ps")
                        for n in range(NQ):
                            nc.tensor.matmul(
                                ps[:, n * 512:(n + 1) * 512],
                                kpb[hb:hb + 64, t, m * 128:(m + 1) * 128],
                                qpb[hb:hb + 64, t, n * 512:(n + 1) * 512],
                                start=True,
                                stop=True,
                            )
                        ex = epool.tile([128, SQ], BF16, name="ex", tag="ex")
                        nc.scalar.activation(ex[:, :], ps[:, :], ACTF.Exp, scale=scale)
                        if dbg and h == 0 and m == 0:
                            nc.sync.dma_start(out=dDEX[:, :], in_=ex[:, :])
                        for n in range(NQ):
                            nc.tensor.matmul(
                                pc[:, n * 512:(n + 1) * 512],
                                vpa[:, m, h, :],
                                ex[:, n * 512:(n + 1) * 512],
                                start=(m == 0),
                                stop=(m == KT8 - 1),
                            )
                        if m % 3 == 2 and nfill < fill_budget and fillers:
                            fillers.pop(0)()
                            nfill += 1
                    # evict ctx rows 0..63 AND the denominator row 64 together
                    cu = smallpool.tile([DH + 1, SQ], F32, name="cu", tag="cu")
                    nc.vector.tensor_copy(cu[:, :], pc[:, :])
                    nc.gpsimd.dma_start(out=cua[hb:hb + 64, :], in_=cu[0:DH, :])
                    # per-head reciprocal chain: DRAM gather-transpose to
                    # [128, 8] -> 128-lane reciprocal -> scatter back ->
                    # partition-broadcast into rb
                    den_d = dpool.tile([1, SQ], F32, name="den_d", tag="den_d")
                    nc.sync.dma_start(out=den_d[:, :], in_=cu[DH:DH + 1, :])
                    rec_d = dpool.tile([1, SQ], F32, name="rec_d", tag="rec_d")
                    dT = smallpool.tile([128, 8], F32, name="dT", tag="dT")
                    nc.sync.dma_start(
                        out=dT[:, :],
                        in_=den_d[0, :].rearrange("(g p) -> p g", p=128),
                    )
                    rT = smallpool.tile([128, 8], F32, name="rT", tag="rT")
                    nc.vector.reciprocal(rT[:, :], dT[:, :])
                    nc.sync.dma_start(
                        out=rec_d[0, :].rearrange("(g p) -> p g", p=128),
                        in_=rT[:, :],
                    )
                    bsrc = bass.AP(
                        tensor=rec_d[0:1, :].tensor,
                        offset=rec_d[0:1, :].offset,
                        ap=[[0, 64], [1, SQ]],
                    )
                    nc.sync.dma_start(out=rb[hb:hb + 64, :], in_=bsrc)
                # normalize + residual: o1[:, t, :] = qp[:, t, :] + cua*rb
                cn = pairpool.tile([128, SQ], F32, name="cn", tag="cn")
                nc.vector.tensor_mul(cn[:, :], cua[:, :], rb[:, :])
                nc.vector.tensor_add(o1[:, t, :], cn[:, :], qp[:, t, :].bitcast(F32))
                if dbg and t == 0:
                    nc.sync.dma_start(out=dDCUA[:, :], in_=cua[:, :])
                    nc.sync.dma_start(out=dDRB[:, :], in_=rb[:, :])

            # ---- emission ----
            for n in range(NQ):
                project(qp, wq, qt, bq[:, 0:1], 0, n, dst2=qpb)
            for n in range(NQ):
                project(kpb, wk, kt, bk[:, 0:1], 0, n)
            for mt in range(KT8):
                project_v(mt)
            attend_pair(0, fill_budget=4)
            attend_pair(1, fill_budget=4)
            attend_pair(2, fill_budget=4)
            attend_pair(3, fill_budget=0)
            assert not fillers, f"{len(fillers)} projection chunks not emitted"

            if dbg:
                nc.sync.dma_start(out=dDQP[:, :, :], in_=qp[:, :, :].bitcast(F32))
                nc.sync.dma_start(out=dDKP[:, :, :], in_=kpb[:, :, :])
                nc.sync.dma_start(out=dDVPA[:, :, :, :], in_=vpa[:, :, :, :])
                nc.sync.dma_start(out=dDO1[:, :, :], in_=o1[:, :, :].bitcast(F32))

            # ---- FFN: out = O1 + bv + relu(WoT.T @ O1 + bo2) ----
            for m in range(MT):
                pf = spool.tile([128, SQ], F32, name="pf", tag="ps")
                for n in range(NQ):
                    for kc in range(KC):
                        nc.tensor.matmul(
                            pf[:, n * 512:(n + 1) * 512],
                            wo[:, kc, m * 128:(m + 1) * 128],
                            o1[:, kc, n * 512:(n + 1) * 512],
                            start=(kc == 0),
                            stop=(kc == KC - 1),
                        )
                rf = smallpool.tile([128, SQ], F32, name="rf", tag="rf")
                nc.vector.tensor_scalar(
                    rf[:, :], pf[:, :], bo2[:, m:m + 1], 0.0, ALU.add, ALU.max
                )
                ot = outpool.tile([128, SQ], F32, name="ot", tag="ot")
                nc.vector.scalar_tensor_tensor(
                    ot[:, :],
                    rf[:, :],
                    bv[:, m:m + 1],
                    o1[:, m, :].bitcast(F32),
                    ALU.add,
                    ALU.add,
                )
                nc.gpsimd.dma_start(
                    out=dOT[m * 128:(m + 1) * 128, :], in_=ot[:, :]
                )

    nc.compile()
    return nc


def _get_nc():
    global _NC
    if _NC is None:
        _NC = _build()
    return _NC


def _prep_inputs(Q, K, Wq, bq, Wk, bk, Wv, bv, Wo, bo):
    Q = np.asarray(Q, dtype=np.float32)
    K = np.asarray(K, dtype=np.float32)
    Wq = np.asarray(Wq, dtype=np.float32)
    Wk = np.asarray(Wk, dtype=np.float32)
    Wv = np.asarray(Wv, dtype=np.float32)
    Wo = np.asarray(Wo, dtype=np.float32)
    bq = np.asarray(bq, dtype=np.float32)
    bk = np.asarray(bk, dtype=np.float32)
    bv = np.asarray(bv, dtype=np.float32)
    bo = np.asarray(bo, dtype=np.float32)

    bo2 = (bo + Wo @ bv).astype(np.float32)

    def btile(b):
        return np.ascontiguousarray(b.reshape(MT, 128).T)

    import ml_dtypes
    bf = ml_dtypes.bfloat16
    shared = {
        "WqT": np.ascontiguousarray(Wq.T),
        "WkT": np.ascontiguousarray(Wk.T).astype(bf),
        "WvT": np.ascontiguousarray(Wv.T).astype(bf),
        "WoT": np.ascontiguousarray(Wo.T),
        "BQ": btile(bq),
        "BK": btile(bk),
        "BO2": btile(bo2),
        "BV": btile(bv),
    }
    in_maps = []
    for c in range(N_CORES):
        m = dict(shared)
        m["QT"] = np.ascontiguousarray(Q[c].T)
        m["KT"] = np.ascontiguousarray(K[c].T).astype(bf)
        in_maps.append(m)
    return in_maps


def run(inputs, trace=False):
    """Run on hardware; returns (output [B,SQ,D] f32, BassKernelResults)."""
    in_maps = _prep_inputs(
        inputs["Q"], inputs["K"], inputs["Wq"], inputs["bq"], inputs["Wk"],
        inputs["bk"], inputs["Wv"], inputs["bv"], inputs["Wo"], inputs["bo"],
    )
    nc = _get_nc()
    res = run_bass_kernel_spmd(
        nc, in_maps, core_ids=list(range(N_CORES)), trace=trace
    )
    out = np.stack(
        [res.results[c]["OT"].T for c in range(N_CORES)], axis=0
    ).astype(np.float32)
    return out, res


def kernel(**inputs):
    nh = inputs.get("num_heads", H)
    assert int(nh) == H, f"kernel hardcodes num_heads={H}, got {nh}"
    out, _ = run(inputs, trace=False)
    return out


if __name__ == "__main__":
    rng = np.random.default_rng(0)
    inputs = {
        "Q": rng.standard_normal((B, SQ, D), dtype=np.float32),
        "K": rng.standard_normal((B, SK, D), dtype=np.float32),
        "Wq": rng.standard_normal((D, D), dtype=np.float32) * 0.04,
        "bq": rng.standard_normal((D,), dtype=np.float32) * 0.04,
        "Wk": rng.standard_normal((D, D), dtype=np.float32) * 0.04,
        "bk": rng.standard_normal((D,), dtype=np.float32) * 0.04,
        "Wv": rng.standard_normal((D, D), dtype=np.float32) * 0.04,
        "bv": rng.standard_normal((D,), dtype=np.float32) * 0.04,
        "Wo": rng.standard_normal((D, D), dtype=np.float32) * 0.04,
        "bo": rng.standard_normal((D,), dtype=np.float32) * 0.04,
        "num_heads": H,
    }
    out = kernel(**inputs)
    print("out", out.shape, out.dtype, float(np.abs(out).max()))
